# revision 15
# baseline (speedup 1.0000x reference)
"""Trainium2 Bass kernel for CNNEmbeddings (one-hot -> 3x conv1d -> concat -> mask -> LayerNorm).

Strategy
--------
The input of each conv is a one-hot encoding of token ids (vocab 6, class 5
dropped), so the three convs (K=3/5/7, Cout=256 each) merge into a single
windowed matmul: for every position, the output 768-vector is

    h[p, :] = sum_d sum_v  onehot(ids[p+d-3] == v) * W_merged[5d+v, :]

i.e. a [35 x 768] table contracted against a windowed one-hot [35 x 128]
stationary tile (exact in bf16; the fp32 weights are split hi/lo into two
bf16 matmuls that accumulate in fp32 PSUM).

LayerNorm stats ride along as 36 extra matmul columns:
  * col 803: mu = onehot_window @ (row_sums(W)/768)
  * cols 768..803: Y = onehot_window @ L  where L = cholesky(W W^T / 768),
    so E[h^2] = |Y|^2, computed with one fused DVE multiply+reduce.
The normalize is a single ScalarE activation pass over PSUM with
per-partition scale (rstd) and bias (-mu * rstd); gamma is folded into the
weight table on the host.

Sharding: data-parallel over batch, 4 rows per core x 8 cores; weights
replicated (tiny). No collectives; host gathers per-core outputs.
"""

import numpy as np
import ml_dtypes

# ---- problem constants (hardcoded per contract) ----
B, L, C = 32, 2048, 768
V, D = 5, 7          # kept vocab classes, window width
KV = D * V           # 35 contraction rows
NCORES = 8
RPC = B // NCORES    # batch rows per core
LP = L + 8           # padded row length (ids at offset 3)
PL = RPC * L         # positions per core
NBLK = PL // 128     # 64 blocks of 128 positions
NC_COLS = C + KV + 1  # 804 = 768 h + 35 chol + 1 mu
EPS = 1e-12
GRP = 2              # blocks per stats group

_PROGRAM_CACHE = {}


def _build_program(use_mask: bool, use_beta: bool, use_gm1: bool, reps: int = 1):
    import concourse.bass as bass
    import concourse.bacc as bacc
    import concourse.tile as tile
    from concourse import mybir

    f32 = mybir.dt.float32
    bf16 = mybir.dt.bfloat16
    AF = mybir.ActivationFunctionType
    OP = mybir.AluOpType

    nc = bacc.Bacc("TRN2", target_bir_lowering=False, debug=False)

    ids5 = nc.declare_dram_parameter("ids5", [V, RPC, LP], bf16, isOutput=False)
    wtbl = nc.declare_dram_parameter("wtbl", [KV, 2, NC_COLS], bf16, isOutput=False)
    vcst = nc.declare_dram_parameter("vcst", [KV, 1], f32, isOutput=False)
    if use_mask:
        mask_in = nc.declare_dram_parameter("mask", [RPC, L], f32, isOutput=False)
    if use_beta:
        beta_in = nc.declare_dram_parameter("beta", [C], f32, isOutput=False)
    if use_gm1:
        gm1_in = nc.declare_dram_parameter("gm1", [C], f32, isOutput=False)
    out_ext = nc.declare_dram_parameter("out", [RPC, L, C], f32, isOutput=True)

    with tile.TileContext(nc) as tc:
        with (
            tc.tile_pool(name="singles", bufs=1) as singles,
            tc.tile_pool(name="osb", bufs=3) as osb_pool,
            tc.tile_pool(name="small", bufs=3) as small,
            tc.tile_pool(name="stats", bufs=4) as stats,
            tc.tile_pool(name="hpsum", bufs=3, space="PSUM") as hpsum,
        ):
            # ---- setup: constant tables ----
            wtbl_sb = singles.tile([KV, 2, NC_COLS], bf16)
            nc.sync.dma_start(out=wtbl_sb, in_=wtbl[:])
            vcst_sb = singles.tile([KV, 1], f32)
            nc.sync.dma_start(out=vcst_sb, in_=vcst[:])
            eps_sb = singles.tile([128, 1], f32)
            nc.vector.memset(eps_sb, float(EPS))

            # ---- build windowed ids then one-hot T [35, PL] (bf16) ----
            # Trep[5d+v, r, l] = ids5[v, r, l + d]   (ids5 starts 3 left-shifted)
            trep = singles.tile([KV, PL], bf16)
            T = singles.tile([KV, PL], bf16)
            ids5_t = ids5.tensor if hasattr(ids5, "tensor") else ids5
            for r in range(RPC):
                src = bass.AP(
                    tensor=ids5_t,
                    offset=r * LP,
                    ap=[[1, D], [RPC * LP, V], [1, L]],
                )
                nc.sync.dma_start(out=trep[:, r * L : (r + 1) * L], in_=src)
                nc.vector.tensor_scalar(
                    out=T[:, r * L : (r + 1) * L],
                    in0=trep[:, r * L : (r + 1) * L],
                    scalar1=vcst_sb,
                    scalar2=None,
                    op0=OP.is_equal,
                )

            if use_mask:
                # m[p, blk] = mask[r, s*128 + p],  blk = r*16 + s
                m_sb = singles.tile([128, NBLK], f32)
                msrc = bass.AP(
                    tensor=mask_in.tensor if hasattr(mask_in, "tensor") else mask_in,
                    offset=0,
                    ap=[[1, 128], [L, RPC], [128, L // 128]],
                )
                nc.sync.dma_start(out=m_sb, in_=msrc)
            if use_beta:
                beta_sb = singles.tile([128, C], f32)
                bsrc = bass.AP(
                    tensor=beta_in.tensor if hasattr(beta_in, "tensor") else beta_in,
                    offset=0,
                    ap=[[0, 128], [1, C]],
                )
                nc.sync.dma_start(out=beta_sb, in_=bsrc)
            if use_gm1:
                gm1_sb = singles.tile([128, C], f32)
                gsrc = bass.AP(
                    tensor=gm1_in.tensor if hasattr(gm1_in, "tensor") else gm1_in,
                    offset=0,
                    ap=[[0, 128], [1, C]],
                )
                nc.sync.dma_start(out=gm1_sb, in_=gsrc)

            # ---- main loop: groups of GRP blocks ----
            # (reps>1 repeats the whole loop for slope-based HW timing)
            for g in range(reps * (NBLK // GRP)):
                g = g % (NBLK // GRP)
                qg = stats.tile([128, GRP], f32, tag="qg")
                mug = stats.tile([128, GRP], f32, tag="mug")
                h_tiles = []
                for j in range(GRP):
                    b = g * GRP + j
                    tsl = T[:, b * 128 : (b + 1) * 128]
                    h = hpsum.tile([128, NC_COLS], f32, tag="h")
                    h_tiles.append(h)
                    # four matmuls: (cols 0:512, cols 512:804) x (hi, lo)
                    nc.tensor.matmul(h[:, 0:512], lhsT=tsl, rhs=wtbl_sb[:, 0, 0:512],
                                     start=True, stop=False)
                    nc.tensor.matmul(h[:, 0:512], lhsT=tsl, rhs=wtbl_sb[:, 1, 0:512],
                                     start=False, stop=True)
                    nc.tensor.matmul(h[:, 512:NC_COLS], lhsT=tsl,
                                     rhs=wtbl_sb[:, 0, 512:NC_COLS],
                                     start=True, stop=False)
                    nc.tensor.matmul(h[:, 512:NC_COLS], lhsT=tsl,
                                     rhs=wtbl_sb[:, 1, 512:NC_COLS],
                                     start=False, stop=True)
                    # extract stats: q = |Y|^2, mu
                    ys = small.tile([128, KV], f32, tag="ys")
                    nc.vector.tensor_copy(out=ys, in_=h[:, C : C + KV])
                    sq = small.tile([128, KV], f32, tag="sq")
                    nc.vector.tensor_mul(out=sq, in0=ys, in1=ys)
                    nc.vector.reduce_sum(
                        out=qg[:, j : j + 1], in_=sq, axis=mybir.AxisListType.X
                    )
                    nc.vector.tensor_copy(out=mug[:, j : j + 1],
                                          in_=h[:, C + KV : C + KV + 1])

                # group stat math on [128, GRP]
                var = stats.tile([128, GRP], f32, tag="var")
                nc.vector.tensor_mul(out=var, in0=mug, in1=mug)
                nc.vector.tensor_tensor(out=var, in0=qg, in1=var, op=OP.subtract)
                if use_mask:
                    mg = m_sb[:, g * GRP : (g + 1) * GRP]
                    m2 = stats.tile([128, GRP], f32, tag="m2")
                    nc.vector.tensor_mul(out=m2, in0=mg, in1=mg)
                    nc.vector.tensor_mul(out=var, in0=var, in1=m2)
                sc = stats.tile([128, GRP], f32, tag="sc")
                nc.scalar.activation(out=sc, in_=var, func=AF.Sqrt, bias=eps_sb)
                nc.vector.reciprocal(out=sc, in_=sc)
                if use_mask:
                    nc.vector.tensor_mul(out=sc, in0=sc, in1=mg)
                nega = stats.tile([128, GRP], f32, tag="nega")
                nc.vector.scalar_tensor_tensor(
                    out=nega, in0=mug, scalar=-1.0, in1=sc,
                    op0=OP.mult, op1=OP.mult,
                )

                # normalize + store
                for j in range(GRP):
                    b = g * GRP + j
                    r, s = b // (L // 128), b % (L // 128)
                    osb = osb_pool.tile([128, C], f32, tag="osb")
                    nc.scalar.activation(
                        out=osb, in_=h_tiles[j][:, 0:C], func=AF.Identity,
                        bias=nega[:, j : j + 1], scale=sc[:, j : j + 1],
                    )
                    if use_gm1:
                        nc.vector.scalar_tensor_tensor(
                            out=osb, in0=gm1_sb, scalar=nega[:, j : j + 1],
                            in1=osb, op0=OP.mult, op1=OP.add,
                        )
                    if use_beta:
                        nc.vector.tensor_add(out=osb, in0=beta_sb, in1=osb)
                    nc.sync.dma_start(
                        out=out_ext[r, s * 128 : (s + 1) * 128, :], in_=osb
                    )

    nc.compile()
    return nc


def _host_prep(input_ids, attention_mask, W3, W5, W7, ln_gamma, ln_beta):
    """Build the merged weight/stat tables and padded id planes."""
    bf = ml_dtypes.bfloat16
    ids = np.asarray(input_ids).astype(np.int64)
    gamma = np.asarray(ln_gamma, dtype=np.float64)
    beta = np.asarray(ln_beta, dtype=np.float64)

    Wm = np.zeros((KV, C), dtype=np.float64)
    for (W, K, c0) in ((np.asarray(W3), 3, 0), (np.asarray(W5), 5, 256),
                       (np.asarray(W7), 7, 512)):
        Wd = W.astype(np.float64)
        for k in range(K):
            d = k - K // 2 + 3
            Wm[V * d : V * d + V, c0 : c0 + 256] = Wd[:, :, k].T

    Wg = Wm * gamma[None, :]
    musum = Wm.sum(axis=1) / float(C)
    G = (Wm @ Wm.T) / float(C)
    Lch = np.linalg.cholesky(G + 1e-14 * np.eye(KV))

    tbl = np.zeros((KV, NC_COLS), dtype=np.float64)
    tbl[:, 0:C] = Wg
    tbl[:, C : C + KV] = Lch
    tbl[:, C + KV] = musum
    tbl32 = tbl.astype(np.float32)
    hi = tbl32.astype(bf)
    lo = (tbl32 - hi.astype(np.float32)).astype(bf)
    wtbl = np.stack([hi, lo], axis=1)  # [35, 2, 804] bf16

    vcst = (np.arange(KV) % V).astype(np.float32).reshape(KV, 1)

    ids_pad = np.full((B, LP), V, dtype=np.int64)  # pad with dropped class
    ids_pad[:, 3 : 3 + L] = ids
    ids_bf = ids_pad.astype(np.float32).astype(bf)

    mask = np.asarray(attention_mask, dtype=np.float32)
    use_mask = not bool(np.all(mask == 1.0))
    use_beta = bool(np.any(beta != 0.0))
    use_gm1 = bool(np.any(gamma != 1.0))

    return wtbl, vcst, ids_bf, mask, use_mask, use_beta, use_gm1, \
        beta.astype(np.float32), (gamma - 1.0).astype(np.float32)


_LAST_EXEC_NS = None
_LAST_RESULTS = None


def kernel(input_ids, attention_mask, W3, W5, W7, ln_gamma, ln_beta):
    global _LAST_EXEC_NS, _LAST_RESULTS
    import os
    from concourse.bass_utils import run_bass_kernel_spmd

    (wtbl, vcst, ids_bf, mask, use_mask, use_beta, use_gm1,
     beta32, gm132) = _host_prep(input_ids, attention_mask, W3, W5, W7,
                                 ln_gamma, ln_beta)

    key = (use_mask, use_beta, use_gm1)
    if key not in _PROGRAM_CACHE:
        _PROGRAM_CACHE[key] = _build_program(*key)
    nc = _PROGRAM_CACHE[key]

    in_maps = []
    for c in range(NCORES):
        rows = ids_bf[c * RPC : (c + 1) * RPC]          # [RPC, LP]
        ids5 = np.broadcast_to(rows[None], (V, RPC, LP)).copy()
        m = {"ids5": ids5, "wtbl": wtbl, "vcst": vcst}
        if use_mask:
            m["mask"] = mask[c * RPC : (c + 1) * RPC].copy()
        if use_beta:
            m["beta"] = beta32
        if use_gm1:
            m["gm1"] = gm132
        in_maps.append(m)

    trace = bool(os.environ.get("CNN_KERNEL_TRACE"))
    res = run_bass_kernel_spmd(nc, in_maps, list(range(NCORES)), trace=trace)
    _LAST_EXEC_NS = res.exec_time_ns
    _LAST_RESULTS = res
    out = np.concatenate(
        [np.asarray(res.results[i]["out"]) for i in range(NCORES)], axis=0
    )
    return out.astype(np.float32)


# revision 17
# speedup vs baseline: 1.2566x; 1.2566x over previous
"""Trainium2 Bass kernel for CNNEmbeddings (one-hot -> 3x conv1d -> concat -> mask -> LayerNorm).

Strategy
--------
The input of each conv is a one-hot encoding of token ids (vocab 6, class 5
dropped), so the three convs (K=3/5/7, Cout=256 each) merge into a single
windowed matmul: for every position, the output 768-vector is

    h[p, :] = sum_d sum_v  onehot(ids[p+d-3] == v) * W_merged[5d+v, :]

i.e. a [35 x 768] table contracted against a windowed one-hot [35 x 128]
stationary tile (exact in bf16; the fp32 weights are split hi/lo into two
bf16 matmuls that accumulate in fp32 PSUM).

LayerNorm stats ride along as 36 extra matmul columns:
  * col 803: mu = onehot_window @ (row_sums(W)/768)
  * cols 768..803: Y = onehot_window @ L  where L = cholesky(W W^T / 768),
    so E[h^2] = |Y|^2, computed with one fused DVE multiply+reduce.
The normalize is a single ScalarE activation pass over PSUM with
per-partition scale (rstd) and bias (-mu * rstd); gamma is folded into the
weight table on the host.

Sharding: data-parallel over batch, 4 rows per core x 8 cores; weights
replicated (tiny). No collectives; host gathers per-core outputs.
"""

import numpy as np
import ml_dtypes

# ---- problem constants (hardcoded per contract) ----
B, L, C = 32, 2048, 768
V, D = 5, 7          # kept vocab classes, window width
KV = D * V           # 35 contraction rows
NCORES = 8
RPC = B // NCORES    # batch rows per core
LP = L + 8           # padded row length (ids at offset 3)
PL = RPC * L         # positions per core
NBLK = PL // 128     # 64 blocks of 128 positions
NC_COLS = C + KV + 1  # 804 = 768 h + 35 chol + 1 mu
EPS = 1e-12
GRP = 2              # blocks per stats group

_PROGRAM_CACHE = {}


def _build_program(use_mask: bool, use_beta: bool, use_gm1: bool, reps: int = 1):
    import concourse.bass as bass
    import concourse.bacc as bacc
    import concourse.tile as tile
    from concourse import mybir

    f32 = mybir.dt.float32
    bf16 = mybir.dt.bfloat16
    AF = mybir.ActivationFunctionType
    OP = mybir.AluOpType

    nc = bacc.Bacc("TRN2", target_bir_lowering=False, debug=False)

    ids5 = nc.declare_dram_parameter("ids5", [V, RPC, LP], bf16, isOutput=False)
    wtbl = nc.declare_dram_parameter("wtbl", [KV, 2, NC_COLS], bf16, isOutput=False)
    vcst = nc.declare_dram_parameter("vcst", [KV, 1], f32, isOutput=False)
    if use_mask:
        mask_in = nc.declare_dram_parameter("mask", [RPC, L], f32, isOutput=False)
    if use_beta:
        beta_in = nc.declare_dram_parameter("beta", [C], f32, isOutput=False)
    if use_gm1:
        gm1_in = nc.declare_dram_parameter("gm1", [C], f32, isOutput=False)
    out_ext = nc.declare_dram_parameter("out", [RPC, L, C], f32, isOutput=True)

    with tile.TileContext(nc) as tc:
        with (
            tc.tile_pool(name="singles", bufs=1) as singles,
            tc.tile_pool(name="osb", bufs=4) as osb_pool,
            tc.tile_pool(name="small", bufs=3) as small,
            tc.tile_pool(name="stats", bufs=4) as stats,
            tc.tile_pool(name="hpsum", bufs=4, space="PSUM") as hpsum,
        ):
            # ---- setup: constant tables ----
            wtbl_sb = singles.tile([KV, 2, NC_COLS], bf16)
            nc.sync.dma_start(out=wtbl_sb, in_=wtbl[:])
            vcst_sb = singles.tile([KV, 1], f32)
            nc.sync.dma_start(out=vcst_sb, in_=vcst[:])
            eps_sb = singles.tile([128, 1], f32)
            nc.vector.memset(eps_sb, float(EPS))

            # ---- build windowed ids then one-hot T [35, PL] (bf16) ----
            # Trep[5d+v, r, l] = ids5[v, r, l + d]   (ids5 starts 3 left-shifted)
            trep = singles.tile([KV, PL], bf16)
            T = singles.tile([KV, PL], bf16)
            ids5_t = ids5.tensor if hasattr(ids5, "tensor") else ids5
            for r in range(RPC):
                src = bass.AP(
                    tensor=ids5_t,
                    offset=r * LP,
                    ap=[[1, D], [RPC * LP, V], [1, L]],
                )
                nc.sync.dma_start(out=trep[:, r * L : (r + 1) * L], in_=src)
                nc.vector.tensor_scalar(
                    out=T[:, r * L : (r + 1) * L],
                    in0=trep[:, r * L : (r + 1) * L],
                    scalar1=vcst_sb,
                    scalar2=None,
                    op0=OP.is_equal,
                )

            if use_mask:
                # m[p, blk] = mask[r, s*128 + p],  blk = r*16 + s
                m_sb = singles.tile([128, NBLK], f32)
                msrc = bass.AP(
                    tensor=mask_in.tensor if hasattr(mask_in, "tensor") else mask_in,
                    offset=0,
                    ap=[[1, 128], [L, RPC], [128, L // 128]],
                )
                nc.sync.dma_start(out=m_sb, in_=msrc)
            if use_beta:
                beta_sb = singles.tile([128, C], f32)
                bsrc = bass.AP(
                    tensor=beta_in.tensor if hasattr(beta_in, "tensor") else beta_in,
                    offset=0,
                    ap=[[0, 128], [1, C]],
                )
                nc.sync.dma_start(out=beta_sb, in_=bsrc)
            if use_gm1:
                gm1_sb = singles.tile([128, C], f32)
                gsrc = bass.AP(
                    tensor=gm1_in.tensor if hasattr(gm1_in, "tensor") else gm1_in,
                    offset=0,
                    ap=[[0, 128], [1, C]],
                )
                nc.sync.dma_start(out=gm1_sb, in_=gsrc)

            # ---- main loop: groups of GRP blocks ----
            # (reps>1 repeats the whole loop for slope-based HW timing)
            for g in range(reps * (NBLK // GRP)):
                g = g % (NBLK // GRP)
                qg = stats.tile([128, GRP], f32, tag="qg")
                mug = stats.tile([128, GRP], f32, tag="mug")
                h_tiles = []
                for j in range(GRP):
                    b = g * GRP + j
                    tsl = T[:, b * 128 : (b + 1) * 128]
                    h = hpsum.tile([128, NC_COLS], f32, tag="h")
                    h_tiles.append(h)
                    # four matmuls: (cols 0:512, cols 512:804) x (hi, lo)
                    nc.tensor.matmul(h[:, 0:512], lhsT=tsl, rhs=wtbl_sb[:, 0, 0:512],
                                     start=True, stop=False)
                    nc.tensor.matmul(h[:, 0:512], lhsT=tsl, rhs=wtbl_sb[:, 1, 0:512],
                                     start=False, stop=True)
                    nc.tensor.matmul(h[:, 512:NC_COLS], lhsT=tsl,
                                     rhs=wtbl_sb[:, 0, 512:NC_COLS],
                                     start=True, stop=False)
                    nc.tensor.matmul(h[:, 512:NC_COLS], lhsT=tsl,
                                     rhs=wtbl_sb[:, 1, 512:NC_COLS],
                                     start=False, stop=True)
                    # extract stats: q = |Y|^2, mu
                    ys = small.tile([128, KV], f32, tag="ys")
                    nc.vector.tensor_copy(out=ys, in_=h[:, C : C + KV])
                    sq = small.tile([128, KV], f32, tag="sq")
                    nc.vector.tensor_mul(out=sq, in0=ys, in1=ys)
                    nc.vector.reduce_sum(
                        out=qg[:, j : j + 1], in_=sq, axis=mybir.AxisListType.X
                    )
                    nc.vector.tensor_copy(out=mug[:, j : j + 1],
                                          in_=h[:, C + KV : C + KV + 1])

                # group stat math on [128, GRP]
                var = stats.tile([128, GRP], f32, tag="var")
                nc.vector.tensor_mul(out=var, in0=mug, in1=mug)
                nc.vector.tensor_tensor(out=var, in0=qg, in1=var, op=OP.subtract)
                if use_mask:
                    mg = m_sb[:, g * GRP : (g + 1) * GRP]
                    m2 = stats.tile([128, GRP], f32, tag="m2")
                    nc.vector.tensor_mul(out=m2, in0=mg, in1=mg)
                    nc.vector.tensor_mul(out=var, in0=var, in1=m2)
                sc = stats.tile([128, GRP], f32, tag="sc")
                nc.scalar.activation(out=sc, in_=var, func=AF.Sqrt, bias=eps_sb)
                nc.vector.reciprocal(out=sc, in_=sc)
                if use_mask:
                    nc.vector.tensor_mul(out=sc, in0=sc, in1=mg)
                nega = stats.tile([128, GRP], f32, tag="nega")
                nc.vector.scalar_tensor_tensor(
                    out=nega, in0=mug, scalar=-1.0, in1=sc,
                    op0=OP.mult, op1=OP.mult,
                )

                # normalize + store (alternate ScalarE / VectorE to split load)
                for j in range(GRP):
                    b = g * GRP + j
                    r, s = b // (L // 128), b % (L // 128)
                    osb = osb_pool.tile([128, C], f32, tag="osb")
                    if j % 2 == 0:
                        nc.scalar.activation(
                            out=osb, in_=h_tiles[j][:, 0:C], func=AF.Identity,
                            bias=nega[:, j : j + 1], scale=sc[:, j : j + 1],
                        )
                    else:
                        nc.vector.tensor_scalar(
                            out=osb, in0=h_tiles[j][:, 0:C],
                            scalar1=mug[:, j : j + 1], scalar2=sc[:, j : j + 1],
                            op0=OP.subtract, op1=OP.mult,
                        )
                    if use_gm1:
                        nc.vector.scalar_tensor_tensor(
                            out=osb, in0=gm1_sb, scalar=nega[:, j : j + 1],
                            in1=osb, op0=OP.mult, op1=OP.add,
                        )
                    if use_beta:
                        nc.vector.tensor_add(out=osb, in0=beta_sb, in1=osb)
                    nc.sync.dma_start(
                        out=out_ext[r, s * 128 : (s + 1) * 128, :], in_=osb
                    )

    nc.compile()
    return nc


def _host_prep(input_ids, attention_mask, W3, W5, W7, ln_gamma, ln_beta):
    """Build the merged weight/stat tables and padded id planes."""
    bf = ml_dtypes.bfloat16
    ids = np.asarray(input_ids).astype(np.int64)
    gamma = np.asarray(ln_gamma, dtype=np.float64)
    beta = np.asarray(ln_beta, dtype=np.float64)

    Wm = np.zeros((KV, C), dtype=np.float64)
    for (W, K, c0) in ((np.asarray(W3), 3, 0), (np.asarray(W5), 5, 256),
                       (np.asarray(W7), 7, 512)):
        Wd = W.astype(np.float64)
        for k in range(K):
            d = k - K // 2 + 3
            Wm[V * d : V * d + V, c0 : c0 + 256] = Wd[:, :, k].T

    Wg = Wm * gamma[None, :]
    musum = Wm.sum(axis=1) / float(C)
    G = (Wm @ Wm.T) / float(C)
    Lch = np.linalg.cholesky(G + 1e-14 * np.eye(KV))

    tbl = np.zeros((KV, NC_COLS), dtype=np.float64)
    tbl[:, 0:C] = Wg
    tbl[:, C : C + KV] = Lch
    tbl[:, C + KV] = musum
    tbl32 = tbl.astype(np.float32)
    hi = tbl32.astype(bf)
    lo = (tbl32 - hi.astype(np.float32)).astype(bf)
    wtbl = np.stack([hi, lo], axis=1)  # [35, 2, 804] bf16

    vcst = (np.arange(KV) % V).astype(np.float32).reshape(KV, 1)

    ids_pad = np.full((B, LP), V, dtype=np.int64)  # pad with dropped class
    ids_pad[:, 3 : 3 + L] = ids
    ids_bf = ids_pad.astype(np.float32).astype(bf)

    mask = np.asarray(attention_mask, dtype=np.float32)
    use_mask = not bool(np.all(mask == 1.0))
    use_beta = bool(np.any(beta != 0.0))
    use_gm1 = bool(np.any(gamma != 1.0))

    return wtbl, vcst, ids_bf, mask, use_mask, use_beta, use_gm1, \
        beta.astype(np.float32), (gamma - 1.0).astype(np.float32)


_LAST_EXEC_NS = None
_LAST_RESULTS = None


def kernel(input_ids, attention_mask, W3, W5, W7, ln_gamma, ln_beta):
    global _LAST_EXEC_NS, _LAST_RESULTS
    import os
    from concourse.bass_utils import run_bass_kernel_spmd

    (wtbl, vcst, ids_bf, mask, use_mask, use_beta, use_gm1,
     beta32, gm132) = _host_prep(input_ids, attention_mask, W3, W5, W7,
                                 ln_gamma, ln_beta)

    key = (use_mask, use_beta, use_gm1)
    if key not in _PROGRAM_CACHE:
        _PROGRAM_CACHE[key] = _build_program(*key)
    nc = _PROGRAM_CACHE[key]

    in_maps = []
    for c in range(NCORES):
        rows = ids_bf[c * RPC : (c + 1) * RPC]          # [RPC, LP]
        ids5 = np.broadcast_to(rows[None], (V, RPC, LP)).copy()
        m = {"ids5": ids5, "wtbl": wtbl, "vcst": vcst}
        if use_mask:
            m["mask"] = mask[c * RPC : (c + 1) * RPC].copy()
        if use_beta:
            m["beta"] = beta32
        if use_gm1:
            m["gm1"] = gm132
        in_maps.append(m)

    trace = bool(os.environ.get("CNN_KERNEL_TRACE"))
    res = run_bass_kernel_spmd(nc, in_maps, list(range(NCORES)), trace=trace)
    _LAST_EXEC_NS = res.exec_time_ns
    _LAST_RESULTS = res
    out = np.concatenate(
        [np.asarray(res.results[i]["out"]) for i in range(NCORES)], axis=0
    )
    return out.astype(np.float32)


# revision 18
# speedup vs baseline: 4.1927x; 3.3365x over previous
"""Trainium2 Bass kernel for CNNEmbeddings (one-hot -> 3x conv1d -> concat -> mask -> LayerNorm).

Strategy
--------
The input of each conv is a one-hot encoding of token ids (vocab 6, class 5
dropped), so the three convs (K=3/5/7, Cout=256 each) merge into a single
windowed matmul: for every position, the output 768-vector is

    h[p, :] = sum_d sum_v  onehot(ids[p+d-3] == v) * W_merged[5d+v, :]

i.e. a [35 x 768] table contracted against a windowed one-hot [35 x 128]
stationary tile (exact in bf16; the fp32 weights are split hi/lo into two
bf16 matmuls that accumulate in fp32 PSUM).

LayerNorm stats ride along as 36 extra matmul columns:
  * col 803: mu = onehot_window @ (row_sums(W)/768)
  * cols 768..803: Y = onehot_window @ L  where L = cholesky(W W^T / 768),
    so E[h^2] = |Y|^2, computed with one fused DVE multiply+reduce.
The normalize is a single ScalarE activation pass over PSUM with
per-partition scale (rstd) and bias (-mu * rstd); gamma is folded into the
weight table on the host.

Sharding: data-parallel over batch, 4 rows per core x 8 cores; weights
replicated (tiny). No collectives; host gathers per-core outputs.
"""

import numpy as np
import ml_dtypes

# ---- problem constants (hardcoded per contract) ----
B, L, C = 32, 2048, 768
V, D = 5, 7          # kept vocab classes, window width
KV = D * V           # 35 contraction rows
NCORES = 8
RPC = B // NCORES    # batch rows per core
LP = L + 8           # padded row length (ids at offset 3)
PL = RPC * L         # positions per core
NBLK = PL // 128     # 64 blocks of 128 positions
NC_COLS = C + KV + 1  # 804 = 768 h + 35 chol + 1 mu
EPS = 1e-12
GRP = 2              # blocks per stats group

_PROGRAM_CACHE = {}


def _build_program(use_mask: bool, use_beta: bool, use_gm1: bool, reps: int = 1):
    import concourse.bass as bass
    import concourse.bacc as bacc
    import concourse.tile as tile
    from concourse import mybir

    f32 = mybir.dt.float32
    bf16 = mybir.dt.bfloat16
    AF = mybir.ActivationFunctionType
    OP = mybir.AluOpType

    nc = bacc.Bacc("TRN2", target_bir_lowering=False, debug=False)

    ids5 = nc.declare_dram_parameter("ids5", [V, RPC, LP], bf16, isOutput=False)
    wtbl = nc.declare_dram_parameter("wtbl", [KV, 2, NC_COLS], bf16, isOutput=False)
    vcst = nc.declare_dram_parameter("vcst", [KV, 1], f32, isOutput=False)
    if use_mask:
        mask_in = nc.declare_dram_parameter("mask", [RPC, L], f32, isOutput=False)
    if use_beta:
        beta_in = nc.declare_dram_parameter("beta", [C], f32, isOutput=False)
    if use_gm1:
        gm1_in = nc.declare_dram_parameter("gm1", [C], f32, isOutput=False)
    out_ext = nc.declare_dram_parameter("out", [RPC, L, C], f32, isOutput=True)

    with tile.TileContext(nc) as tc:
        with (
            tc.tile_pool(name="singles", bufs=1) as singles,
            tc.tile_pool(name="osb", bufs=4) as osb_pool,
            tc.tile_pool(name="small", bufs=3) as small,
            tc.tile_pool(name="stats", bufs=4) as stats,
            tc.tile_pool(name="hpsum", bufs=4, space="PSUM") as hpsum,
        ):
            # ---- setup: constant tables ----
            wtbl_sb = singles.tile([KV, 2, NC_COLS], bf16)
            nc.sync.dma_start(out=wtbl_sb, in_=wtbl[:])
            vcst_sb = singles.tile([KV, 1], f32)
            nc.sync.dma_start(out=vcst_sb, in_=vcst[:])
            eps_sb = singles.tile([128, 1], f32)
            nc.vector.memset(eps_sb, float(EPS))

            # ---- build windowed ids then one-hot T [35, PL] (bf16) ----
            # Trep[5d+v, r, l] = ids5[v, r, l + d]   (ids5 starts 3 left-shifted)
            trep = singles.tile([KV, PL], bf16)
            T = singles.tile([KV, PL], bf16)
            ids5_t = ids5.tensor if hasattr(ids5, "tensor") else ids5
            for r in range(RPC):
                src = bass.AP(
                    tensor=ids5_t,
                    offset=r * LP,
                    ap=[[1, D], [RPC * LP, V], [1, L]],
                )
                nc.sync.dma_start(out=trep[:, r * L : (r + 1) * L], in_=src)
                nc.vector.tensor_scalar(
                    out=T[:, r * L : (r + 1) * L],
                    in0=trep[:, r * L : (r + 1) * L],
                    scalar1=vcst_sb,
                    scalar2=None,
                    op0=OP.is_equal,
                )

            if use_mask:
                # m[p, blk] = mask[r, s*128 + p],  blk = r*16 + s
                m_sb = singles.tile([128, NBLK], f32)
                msrc = bass.AP(
                    tensor=mask_in.tensor if hasattr(mask_in, "tensor") else mask_in,
                    offset=0,
                    ap=[[1, 128], [L, RPC], [128, L // 128]],
                )
                nc.sync.dma_start(out=m_sb, in_=msrc)
            if use_beta:
                beta_sb = singles.tile([128, C], f32)
                bsrc = bass.AP(
                    tensor=beta_in.tensor if hasattr(beta_in, "tensor") else beta_in,
                    offset=0,
                    ap=[[0, 128], [1, C]],
                )
                nc.sync.dma_start(out=beta_sb, in_=bsrc)
            if use_gm1:
                gm1_sb = singles.tile([128, C], f32)
                gsrc = bass.AP(
                    tensor=gm1_in.tensor if hasattr(gm1_in, "tensor") else gm1_in,
                    offset=0,
                    ap=[[0, 128], [1, C]],
                )
                nc.sync.dma_start(out=gm1_sb, in_=gsrc)

            # ---- main loop: groups of GRP blocks ----
            # (reps>1 repeats the whole loop for slope-based HW timing)
            for g in range(reps * (NBLK // GRP)):
                g = g % (NBLK // GRP)
                qg = stats.tile([128, GRP], f32, tag="qg")
                mug = stats.tile([128, GRP], f32, tag="mug")
                h_tiles = []
                for j in range(GRP):
                    b = g * GRP + j
                    tsl = T[:, b * 128 : (b + 1) * 128]
                    h = hpsum.tile([128, NC_COLS], f32, tag="h")
                    h_tiles.append(h)
                    # four matmuls: (cols 0:512, cols 512:804) x (hi, lo)
                    nc.tensor.matmul(h[:, 0:512], lhsT=tsl, rhs=wtbl_sb[:, 0, 0:512],
                                     start=True, stop=False)
                    nc.tensor.matmul(h[:, 0:512], lhsT=tsl, rhs=wtbl_sb[:, 1, 0:512],
                                     start=False, stop=True)
                    nc.tensor.matmul(h[:, 512:NC_COLS], lhsT=tsl,
                                     rhs=wtbl_sb[:, 0, 512:NC_COLS],
                                     start=True, stop=False)
                    nc.tensor.matmul(h[:, 512:NC_COLS], lhsT=tsl,
                                     rhs=wtbl_sb[:, 1, 512:NC_COLS],
                                     start=False, stop=True)
                    # extract stats: q = |Y|^2 (one ACT square+accumulate), mu
                    sq = small.tile([128, KV], f32, tag="sq")
                    nc.scalar.activation(out=sq, in_=h[:, C : C + KV],
                                         func=AF.Square,
                                         accum_out=qg[:, j : j + 1])
                    nc.vector.tensor_copy(out=mug[:, j : j + 1],
                                          in_=h[:, C + KV : C + KV + 1])

                # group stat math on [128, GRP]
                var = stats.tile([128, GRP], f32, tag="var")
                nc.vector.tensor_mul(out=var, in0=mug, in1=mug)
                nc.vector.tensor_tensor(out=var, in0=qg, in1=var, op=OP.subtract)
                if use_mask:
                    mg = m_sb[:, g * GRP : (g + 1) * GRP]
                    m2 = stats.tile([128, GRP], f32, tag="m2")
                    nc.vector.tensor_mul(out=m2, in0=mg, in1=mg)
                    nc.vector.tensor_mul(out=var, in0=var, in1=m2)
                sc = stats.tile([128, GRP], f32, tag="sc")
                nc.scalar.activation(out=sc, in_=var, func=AF.Sqrt, bias=eps_sb)
                nc.vector.reciprocal(out=sc, in_=sc)
                if use_mask:
                    nc.vector.tensor_mul(out=sc, in0=sc, in1=mg)
                nega = stats.tile([128, GRP], f32, tag="nega")
                nc.vector.scalar_tensor_tensor(
                    out=nega, in0=mug, scalar=-1.0, in1=sc,
                    op0=OP.mult, op1=OP.mult,
                )

                # normalize + store (alternate ScalarE / VectorE to split load)
                for j in range(GRP):
                    b = g * GRP + j
                    r, s = b // (L // 128), b % (L // 128)
                    osb = osb_pool.tile([128, C], f32, tag="osb")
                    if j % 2 == 0:
                        nc.scalar.activation(
                            out=osb, in_=h_tiles[j][:, 0:C], func=AF.Identity,
                            bias=nega[:, j : j + 1], scale=sc[:, j : j + 1],
                        )
                    else:
                        nc.vector.tensor_scalar(
                            out=osb, in0=h_tiles[j][:, 0:C],
                            scalar1=mug[:, j : j + 1], scalar2=sc[:, j : j + 1],
                            op0=OP.subtract, op1=OP.mult,
                        )
                    if use_gm1:
                        nc.vector.scalar_tensor_tensor(
                            out=osb, in0=gm1_sb, scalar=nega[:, j : j + 1],
                            in1=osb, op0=OP.mult, op1=OP.add,
                        )
                    if use_beta:
                        nc.vector.tensor_add(out=osb, in0=beta_sb, in1=osb)
                    nc.sync.dma_start(
                        out=out_ext[r, s * 128 : (s + 1) * 128, :], in_=osb
                    )

    nc.compile()
    return nc


def _host_prep(input_ids, attention_mask, W3, W5, W7, ln_gamma, ln_beta):
    """Build the merged weight/stat tables and padded id planes."""
    bf = ml_dtypes.bfloat16
    ids = np.asarray(input_ids).astype(np.int64)
    gamma = np.asarray(ln_gamma, dtype=np.float64)
    beta = np.asarray(ln_beta, dtype=np.float64)

    Wm = np.zeros((KV, C), dtype=np.float64)
    for (W, K, c0) in ((np.asarray(W3), 3, 0), (np.asarray(W5), 5, 256),
                       (np.asarray(W7), 7, 512)):
        Wd = W.astype(np.float64)
        for k in range(K):
            d = k - K // 2 + 3
            Wm[V * d : V * d + V, c0 : c0 + 256] = Wd[:, :, k].T

    Wg = Wm * gamma[None, :]
    musum = Wm.sum(axis=1) / float(C)
    G = (Wm @ Wm.T) / float(C)
    Lch = np.linalg.cholesky(G + 1e-14 * np.eye(KV))

    tbl = np.zeros((KV, NC_COLS), dtype=np.float64)
    tbl[:, 0:C] = Wg
    tbl[:, C : C + KV] = Lch
    tbl[:, C + KV] = musum
    tbl32 = tbl.astype(np.float32)
    hi = tbl32.astype(bf)
    lo = (tbl32 - hi.astype(np.float32)).astype(bf)
    wtbl = np.stack([hi, lo], axis=1)  # [35, 2, 804] bf16

    vcst = (np.arange(KV) % V).astype(np.float32).reshape(KV, 1)

    ids_pad = np.full((B, LP), V, dtype=np.int64)  # pad with dropped class
    ids_pad[:, 3 : 3 + L] = ids
    ids_bf = ids_pad.astype(np.float32).astype(bf)

    mask = np.asarray(attention_mask, dtype=np.float32)
    use_mask = not bool(np.all(mask == 1.0))
    use_beta = bool(np.any(beta != 0.0))
    use_gm1 = bool(np.any(gamma != 1.0))

    return wtbl, vcst, ids_bf, mask, use_mask, use_beta, use_gm1, \
        beta.astype(np.float32), (gamma - 1.0).astype(np.float32)


_LAST_EXEC_NS = None
_LAST_RESULTS = None


def kernel(input_ids, attention_mask, W3, W5, W7, ln_gamma, ln_beta):
    global _LAST_EXEC_NS, _LAST_RESULTS
    import os
    from concourse.bass_utils import run_bass_kernel_spmd

    (wtbl, vcst, ids_bf, mask, use_mask, use_beta, use_gm1,
     beta32, gm132) = _host_prep(input_ids, attention_mask, W3, W5, W7,
                                 ln_gamma, ln_beta)

    key = (use_mask, use_beta, use_gm1)
    if key not in _PROGRAM_CACHE:
        _PROGRAM_CACHE[key] = _build_program(*key)
    nc = _PROGRAM_CACHE[key]

    in_maps = []
    for c in range(NCORES):
        rows = ids_bf[c * RPC : (c + 1) * RPC]          # [RPC, LP]
        ids5 = np.broadcast_to(rows[None], (V, RPC, LP)).copy()
        m = {"ids5": ids5, "wtbl": wtbl, "vcst": vcst}
        if use_mask:
            m["mask"] = mask[c * RPC : (c + 1) * RPC].copy()
        if use_beta:
            m["beta"] = beta32
        if use_gm1:
            m["gm1"] = gm132
        in_maps.append(m)

    trace = bool(os.environ.get("CNN_KERNEL_TRACE"))
    res = run_bass_kernel_spmd(nc, in_maps, list(range(NCORES)), trace=trace)
    _LAST_EXEC_NS = res.exec_time_ns
    _LAST_RESULTS = res
    out = np.concatenate(
        [np.asarray(res.results[i]["out"]) for i in range(NCORES)], axis=0
    )
    return out.astype(np.float32)
